# revision 16
# baseline (speedup 1.0000x reference)
"""Trainium2 Bass kernel for nn_GCNNet (3-layer GCNConv+BN+ReLU, JK concat),
distributed over 8 NeuronCores.

Strategy (graph parallel): nodes are partitioned across the 8 cores
(round-robin by degree, then bin-packed into 128-node tiles balancing
per-tile in-edge counts).  Each edge is assigned to the core that owns its
destination node.  Per layer:
  m = (o @ W_l) * dinv          computed node-sharded
  all-gather of m -> per-core DRAM replica (layer 1's m is precomputed on
  the host from x and fed as an input, so layer 1 needs no collective)
  raw_agg[d] = sum_{e:dst=d} m[src_e]   via dma_gather (128 rows/chunk) and
                                        a one-hot selection matmul on PE that
                                        performs the segmented sum in PSUM;
                                        self-loops are folded in as one
                                        diagonal matmul per tile (no gather
                                        rows spent on them)
  agg = raw_agg * dinv[dst]     (GCNConv bias cancels inside BatchNorm)
  BN stats via a tiny AllGather; o' = relu(a*agg + b) on the scalar engine.

kernel(**inputs) takes the FULL inputs and returns the FULL [N, 512] output.
"""

import ml_dtypes as _ml_dtypes
import numpy as np

import concourse.bacc as bacc
import concourse.bass as bass
import concourse.mybir as mybir
import concourse.tile as tile
from concourse.library_config import mlp as mlp_library

F32 = mybir.dt.float32
I16 = mybir.dt.int16
BF16 = mybir.dt.bfloat16
AX = mybir.AxisListType
OP = mybir.AluOpType
ACTF = mybir.ActivationFunctionType

C = 8


# ----------------------------------------------------------------------------
# Host preprocessing
# ----------------------------------------------------------------------------

class Cfg:
    pass


def preprocess(x, edge_index, Ws, gammas, betas, G=1, eps=1e-5):
    """Build per-core device inputs + schedule constants from the graph."""
    N, D = x.shape
    assert D == 128
    L = Ws.shape[0]
    E = edge_index.shape[1]

    NPC = (N + C - 1) // C          # nodes per core (max)
    NT = (NPC + 127) // 128         # tiles per core
    NTP = NT * 128                  # slots per core
    SLOTS = C * NTP
    HALF = (C // 2) * NTP
    assert HALF < 32768 and (SLOTS - HALF) < 32768

    src = edge_index[0].astype(np.int64)
    dst = edge_index[1].astype(np.int64)

    deg = np.bincount(dst, minlength=N).astype(np.float64) + 1.0
    dinv = (1.0 / np.sqrt(deg)).astype(np.float32)

    # --- node -> core assignment: deal round-robin in degree order -------
    order = np.argsort(-deg, kind="stable")
    core_of = np.empty(N, np.int64)
    core_of[order] = np.arange(N) % C

    # --- per-node A/B in-degree (A = src owned by cores < C/2); no self -
    srcA = core_of[src] < (C // 2)
    dA = np.bincount(dst[srcA], minlength=N)
    dB = np.bincount(dst[~srcA], minlength=N)

    # --- per-core tile packing (greedy balance of dA and dB) -------------
    tile_of = np.empty(N, np.int64)
    pos_of = np.empty(N, np.int64)
    for c in range(C):
        nodes = np.where(core_of == c)[0]
        nodes = nodes[np.argsort(-(dA[nodes] + dB[nodes]), kind="stable")]
        loadA = np.zeros(NT, np.float64)
        loadB = np.zeros(NT, np.float64)
        cnt = np.zeros(NT, np.int64)
        tA = max(dA[nodes].sum() / NT, 1.0)
        tB = max(dB[nodes].sum() / NT, 1.0)
        for v in nodes:
            score = np.maximum((loadA + dA[v]) / tA, (loadB + dB[v]) / tB)
            score = score + np.where(cnt >= 128, 1e18, 0.0)
            t = int(np.argmin(score))
            tile_of[v] = t
            pos_of[v] = cnt[t]
            cnt[t] += 1
            loadA[t] += dA[v]
            loadB[t] += dB[v]

    slot_of = core_of * NTP + tile_of * 128 + pos_of
    node_of_slot = np.full(SLOTS, -1, np.int64)
    node_of_slot[slot_of] = np.arange(N)

    # --- edge arrays (no self-loops), assigned to dst (core,tile) --------
    e_srcslot = slot_of[src]
    e_grpB = (e_srcslot >= HALF).astype(np.int64)
    e_core = core_of[dst]
    e_tile = tile_of[dst]
    e_pos = pos_of[dst]

    # per-core, per-(tile,grp) edge lists; chunk counts = max across cores
    per_core_edges = []
    ch_tg_max = np.zeros(NT * 2, np.int64)
    for r in range(C):
        mine = np.where(e_core == r)[0]
        key = e_tile[mine] * 2 + e_grpB[mine]
        eo = np.argsort(key, kind="stable")
        key_s = key[eo]
        row_s = np.where(e_grpB[mine][eo] == 1,
                         e_srcslot[mine][eo] - HALF, e_srcslot[mine][eo])
        pos_s = e_pos[mine][eo]
        cnt_tg = np.bincount(key_s, minlength=NT * 2)
        ch_tg_max = np.maximum(ch_tg_max, (cnt_tg + 127) // 128)
        per_core_edges.append((cnt_tg, row_s, pos_s))

    LA_t = ch_tg_max[0::2].copy()
    LB_t = ch_tg_max[1::2].copy()
    CH_t = LA_t + LB_t
    TOTCH = int(CH_t.sum())
    dstb_off_t = np.concatenate([[0], np.cumsum(CH_t)[:-1]]).astype(np.int64)

    # call plan: groups of G tiles x {A,B}; variable chunks per tile
    groups = [list(range(g, min(g + G, NT))) for g in range(0, NT, G)]
    call_plan = []       # (grp, tiles, (chunks per tile), idx_off, n_idx)
    off = 0
    for tiles_g in groups:
        for grp in (0, 1):
            chs = tuple(int(ch_tg_max[2 * t + grp]) for t in tiles_g)
            n = sum(chs) * 128
            call_plan.append((grp, tuple(tiles_g), chs, off, n))
            off += n
    NIDX = off
    assert NIDX == TOTCH * 128
    GMAX = max(sum(cp[2]) for cp in call_plan)

    # gbuf column index of chunk cc of tile t (within its call):
    # host-side map: (t, cc) -> (call_id, col)
    col_of = {}
    for ci, (grp, tiles_g, chs, off0, n) in enumerate(call_plan):
        col = 0
        for t, nch in zip(tiles_g, chs):
            for i in range(nch):
                cc = i if grp == 0 else int(LA_t[t]) + i
                col_of[(t, cc)] = (ci, col)
                col += 1

    # per-core tables ------------------------------------------------------
    per_core = []
    for r in range(C):
        cnt_tg, row_s, pos_s = per_core_edges[r]
        starts = np.concatenate([[0], np.cumsum(cnt_tg)[:-1]])

        idx_flat = np.zeros(NIDX, np.int64)
        dstb = np.full((TOTCH, 128), -1.0, np.float32)
        ioff = 0
        for grp, tiles_g, chs, off0, n in call_plan:
            for t, nch in zip(tiles_g, chs):
                tg = 2 * t + grp
                cnt = int(cnt_tg[tg])
                s0 = int(starts[tg])
                room = nch * 128
                seg = np.zeros(room, np.int64)
                seg[:cnt] = row_s[s0 : s0 + cnt]
                idx_flat[ioff : ioff + room] = seg
                col0 = int(dstb_off_t[t]) + (0 if grp == 0 else int(LA_t[t]))
                dseg = np.full(room, -1.0, np.float32)
                dseg[:cnt] = pos_s[s0 : s0 + cnt].astype(np.float32)
                dstb[col0 : col0 + nch, :] = dseg.reshape(nch, 128)
                ioff += room
        assert ioff == NIDX

        idx16w = idx_flat.reshape(-1, 16).T.astype(np.int16)
        idx16w = np.tile(idx16w, (8, 1))          # [128, NIDX/16]

        nodes_r = np.where(core_of == r)[0]
        cols_r = tile_of[nodes_r] * 128 + pos_of[nodes_r]
        dinv_cols = np.zeros(NTP, np.float32)
        dinv_cols[cols_r] = dinv[nodes_r]

        dinv_nm = np.zeros((128, NT), np.float32)
        dinv_nm[pos_of[nodes_r], tile_of[nodes_r]] = dinv[nodes_r]

        # self-loop diagonal: m already carries dinv[src] and the drain
        # multiplies dinv[dst], so the diagonal is pure 0/1 (1 = real node)
        Dd = np.zeros((128, NT, 128), np.float32)
        Dd[pos_of[nodes_r], tile_of[nodes_r], pos_of[nodes_r]] = 1.0

        d = {
            "dinv_fm": np.tile(dinv_cols[None, :], (128, 1)),
            "dinv_nm": dinv_nm,
            "idx16": np.ascontiguousarray(idx16w),
            "dstb": np.ascontiguousarray(
                dstb.T.astype(_ml_dtypes.bfloat16)),
            "iota": np.tile(np.arange(128, dtype=np.float32)[None, :],
                            (128, 1)).astype(_ml_dtypes.bfloat16),
            "Dd": np.ascontiguousarray(
                Dd.reshape(128, NT * 128).astype(_ml_dtypes.bfloat16)),
            "Ws": Ws.astype(np.float32),
            "gammaT": np.ascontiguousarray(gammas.T.astype(np.float32)),
            "betaT": np.ascontiguousarray(betas.T.astype(np.float32)),
        }
        per_core.append(d)

    # --- host-precomputed layer-1 messages -------------------------------
    # m1[v] = (x[v] @ W0) * dinv[v]; laid out by slot, replicated per core.
    m1 = (x.astype(np.float32) @ Ws[0].astype(np.float32)) * dinv[:, None]
    m1_full = np.zeros((SLOTS, 128), np.float32)
    m1_full[slot_of] = m1
    m1_full = m1_full.astype(_ml_dtypes.bfloat16)
    for r in range(C):
        nodes_r = np.where(core_of == r)[0]
        m1_nm = np.zeros((128, NT, 128), np.float32)
        m1_nm[pos_of[nodes_r], tile_of[nodes_r], :] = m1[nodes_r]
        per_core[r]["m1_full"] = m1_full
        per_core[r]["m1_nm"] = np.ascontiguousarray(
            m1_nm.reshape(128, NT * 128).astype(_ml_dtypes.bfloat16))

    cfg = Cfg()
    cfg.N, cfg.D, cfg.L, cfg.E = N, D, L, E
    cfg.NPC, cfg.NT, cfg.NTP, cfg.SLOTS, cfg.HALF = NPC, NT, NTP, SLOTS, HALF
    cfg.LA_t, cfg.LB_t, cfg.CH_t, cfg.TOTCH = LA_t, LB_t, CH_t, TOTCH
    cfg.NIDX = NIDX
    cfg.GMAX = GMAX
    cfg.groups = groups
    cfg.call_plan = call_plan
    cfg.dstb_off_t = dstb_off_t
    cfg.col_of = col_of
    cfg.eps = eps
    cfg.core_of = core_of
    cfg.tile_of = tile_of
    cfg.pos_of = pos_of
    cfg.slot_of = slot_of
    cfg.node_of_slot = node_of_slot
    return cfg, per_core


def assemble_output(cfg, x, core_outs):
    """core_outs: list of o_out arrays [L,128,NTP] per core -> [N, (L+1)*128]."""
    N, L, NTP = cfg.N, cfg.L, cfg.NTP
    out = np.empty((N, (L + 1) * 128), np.float32)
    out[:, :128] = x
    for c in range(C):
        slots = cfg.node_of_slot[c * NTP : (c + 1) * NTP]
        valid = slots >= 0
        nodes = slots[valid]
        for l in range(L):
            out[nodes, (l + 1) * 128 : (l + 2) * 128] = \
                core_outs[c][l][:, valid].T
    return out


# ----------------------------------------------------------------------------
# Bass kernel
# ----------------------------------------------------------------------------

def build_nc(cfg):
    NT, NTP = cfg.NT, cfg.NTP
    SLOTS, HALF, L = cfg.SLOTS, cfg.HALF, cfg.L
    TOTCH, NIDX = cfg.TOTCH, cfg.NIDX
    LA_t, CH_t = cfg.LA_t, cfg.CH_t
    GMAX = cfg.GMAX
    IDXW = NIDX // 16

    nc = bacc.Bacc("TRN2", target_bir_lowering=False, num_devices=C)

    dinv_fm_t = nc.dram_tensor("dinv_fm", [128, NTP], F32,
                               kind="ExternalInput")
    dinv_nm_t = nc.dram_tensor("dinv_nm", [128, NT], F32,
                               kind="ExternalInput")
    idx16_t = nc.dram_tensor("idx16", [128, IDXW], I16, kind="ExternalInput")
    dstb_t = nc.dram_tensor("dstb", [128, TOTCH], BF16, kind="ExternalInput")
    iota_t = nc.dram_tensor("iota", [128, 128], BF16, kind="ExternalInput")
    Dd_t = nc.dram_tensor("Dd", [128, NT * 128], BF16, kind="ExternalInput")
    m1_full_t = nc.dram_tensor("m1_full", [SLOTS, 128], BF16,
                               kind="ExternalInput")
    m1_nm_t = nc.dram_tensor("m1_nm", [128, NT * 128], BF16,
                             kind="ExternalInput")
    Ws_t = nc.dram_tensor("Ws", [L, 128, 128], F32, kind="ExternalInput")
    gammaT_t = nc.dram_tensor("gammaT", [128, L], F32, kind="ExternalInput")
    betaT_t = nc.dram_tensor("betaT", [128, L], F32, kind="ExternalInput")
    o_out_t = nc.dram_tensor("o_out", [L, 128, NTP], F32,
                             kind="ExternalOutput")

    with tile.TileContext(nc) as tc:
        with (
            tc.tile_pool(name="persist", bufs=1) as pp,
            tc.tile_pool(name="gath", bufs=3) as gp,
            tc.tile_pool(name="work", bufs=3) as wp,
            tc.tile_pool(name="psum", bufs=4, space="PSUM") as psp,
            tc.tile_pool(name="dram", bufs=1, space="DRAM") as dp,
        ):
            o_fm = pp.tile([128, NTP], F32)
            m_sb = pp.tile([128, NT, 128], BF16)
            dinv_fm = pp.tile([128, NTP], F32)
            dinv_nm = pp.tile([128, NT], F32)
            idx16 = pp.tile([128, IDXW], I16)
            dstb = pp.tile([128, TOTCH], BF16)
            iota = pp.tile([128, 128], BF16)
            Dd = pp.tile([128, NT, 128], BF16)
            Wt = pp.tile([128, L, 128], F32)
            gammaT = pp.tile([128, L], F32)
            betaT = pp.tile([128, L], F32)
            stat = pp.tile([128, 2], F32)
            stat2 = pp.tile([128, 2, 2], F32)
            statr = pp.tile([128, 2], F32)
            statg = pp.tile([128, 2, 8], F32)
            prm = pp.tile([128, 8], F32)

            m_slice_ds = [dp.tile([NTP, 128], BF16, name=f"m_slice_{l}")
                          for l in range(1, L)]
            m_full_ds = [dp.tile([SLOTS, 128], BF16, addr_space="Shared",
                                 name=f"m_full_{l}") for l in range(1, L)]
            stat_in_ds = [dp.tile([128, 2], F32, name=f"stat_in_{l}")
                          for l in range(L)]
            stat_out_ds = [dp.tile([128 * 8, 2], F32, addr_space="Shared",
                                   name=f"stat_out_{l}") for l in range(L)]

            # --- load phase ---------------------------------------------
            nc.gpsimd.load_library(mlp_library)
            nc.sync.dma_start(dinv_fm[:], dinv_fm_t[:])
            nc.sync.dma_start(dinv_nm[:], dinv_nm_t[:])
            nc.sync.dma_start(idx16[:], idx16_t[:])
            nc.sync.dma_start(dstb[:], dstb_t[:])
            nc.sync.dma_start(iota[:], iota_t[:])
            nc.sync.dma_start(Dd[:].rearrange("p t f -> p (t f)"), Dd_t[:])
            nc.sync.dma_start(m_sb[:].rearrange("p t f -> p (t f)"),
                              m1_nm_t[:])
            nc.sync.dma_start(Wt[:], Ws_t[:].rearrange("l k f -> k l f"))
            nc.sync.dma_start(gammaT[:], gammaT_t[:])
            nc.sync.dma_start(betaT[:], betaT_t[:])

            inv_n = 1.0 / float(cfg.N)

            for l in range(L):
                if l == 0:
                    m_full_ap = m1_full_t
                else:
                    m_slice_d = m_slice_ds[l - 1]
                    m_full_d = m_full_ds[l - 1]
                    # m = (o @ W_l) * dinv  (node-major blocks)
                    for b in range(NT):
                        pm = psp.tile([128, 128], F32, name="pm")
                        nc.tensor.matmul(
                            pm[:], lhsT=o_fm[:, b * 128 : (b + 1) * 128],
                            rhs=Wt[:, l, :], start=True, stop=True)
                        nc.scalar.activation(
                            m_sb[:, b, :], pm[:], ACTF.Copy,
                            scale=dinv_nm[:, b : b + 1])
                    # m -> DRAM (node-major rows) in halves, then all-gather
                    NTH = NT // 2
                    nc.sync.dma_start(
                        m_slice_d[0 : NTH * 128, :].rearrange(
                            "(b p) f -> p b f", p=128),
                        m_sb[:, 0:NTH, :])
                    nc.sync.dma_start(
                        m_slice_d[NTH * 128 :, :].rearrange(
                            "(b p) f -> p b f", p=128),
                        m_sb[:, NTH:, :])
                    nc.gpsimd.collective_compute(
                        "AllGather", OP.bypass,
                        replica_groups=[list(range(C))],
                        ins=[m_slice_d[:]], outs=[m_full_d[:]])
                    m_full_ap = m_full_d

                # --- gather + aggregate ---------------------------------
                gbufs = {}
                for ci, (grp, tiles_g, chs, off0, n) in \
                        enumerate(cfg.call_plan):
                    if n == 0:
                        continue
                    gl = gp.tile([128, GMAX, 128], BF16, name="gbuf")
                    src_ap = (m_full_ap[0:HALF, :] if grp == 0
                              else m_full_ap[HALF:SLOTS, :])
                    nc.gpsimd.dma_gather(
                        gl[:, 0 : sum(chs), :], src_ap,
                        idx16[:, off0 // 16 : (off0 + n) // 16],
                        n, n, 128, single_packet=False)
                    gbufs[ci] = gl

                    # aggregate tiles whose chunks are complete: both the
                    # A call (even ci) and B call (odd ci) for this tile
                    # group have been issued once ci is odd.
                    if ci % 2 == 1:
                        for t in cfg.call_plan[ci][1]:
                            cht = int(CH_t[t])
                            off_t = int(cfg.dstb_off_t[t])
                            pa = psp.tile([128, 128], F32, name="pa")
                            if cht > 0:
                                S = wp.tile([128, cht, 128], BF16, name="S")
                                nc.vector.tensor_tensor(
                                    S[:],
                                    dstb[:, off_t : off_t + cht]
                                        .unsqueeze(2)
                                        .to_broadcast([128, cht, 128]),
                                    iota[:].unsqueeze(1)
                                        .to_broadcast([128, cht, 128]),
                                    OP.is_equal)
                                for cc in range(cht):
                                    cci, col = cfg.col_of[(t, cc)]
                                    nc.tensor.matmul(
                                        pa[:], lhsT=gbufs[cci][:, col, :],
                                        rhs=S[:, cc, :],
                                        start=(cc == 0), stop=False)
                            nc.tensor.matmul(
                                pa[:], lhsT=m_sb[:, t, :], rhs=Dd[:, t, :],
                                start=(cht == 0), stop=True)
                            nc.vector.tensor_tensor(
                                o_fm[:, t * 128 : (t + 1) * 128], pa[:],
                                dinv_fm[:, t * 128 : (t + 1) * 128], OP.mult)

                # --- BN stats (biased, over all N real nodes) -----------
                stat_in_d = stat_in_ds[l]
                stat_out_d = stat_out_ds[l]
                NTH = NT // 2
                m_flat = m_sb[:].rearrange("p t f -> p (t f)")
                for h, (c0, c1) in enumerate(((0, NTH * 128),
                                              (NTH * 128, NTP))):
                    nc.vector.tensor_reduce(
                        stat2[:, h, 0:1], o_fm[:, c0:c1], axis=AX.X,
                        op=OP.add)
                    nc.scalar.square(m_flat[:, c0:c1], o_fm[:, c0:c1])
                    nc.vector.tensor_reduce(
                        stat2[:, h, 1:2], m_flat[:, c0:c1], axis=AX.X,
                        op=OP.add)
                nc.vector.tensor_tensor(
                    stat[:, :], stat2[:, 0, :], stat2[:, 1, :], OP.add)
                nc.sync.dma_start(stat_in_d[:], stat[:])
                nc.gpsimd.collective_compute(
                    "AllGather", OP.bypass,
                    replica_groups=[list(range(C))],
                    ins=[stat_in_d[:]], outs=[stat_out_d[:]])
                nc.sync.dma_start(
                    statg[:],
                    stat_out_d[:].rearrange("(r p) j -> p j r", p=128))
                nc.vector.tensor_tensor(statg[:, :, 0:4], statg[:, :, 0:4],
                                        statg[:, :, 4:8], OP.add)
                nc.vector.tensor_tensor(statg[:, :, 0:2], statg[:, :, 0:2],
                                        statg[:, :, 2:4], OP.add)
                nc.vector.tensor_tensor(statr[:, 0:1], statg[:, 0, 0:1],
                                        statg[:, 0, 1:2], OP.add)
                nc.vector.tensor_tensor(statr[:, 1:2], statg[:, 1, 0:1],
                                        statg[:, 1, 1:2], OP.add)

                # mu = S1/N; var = S2/N - mu^2; a = gamma*rsqrt(var+eps);
                # b = beta - mu*a
                mu = prm[:, 0:1]
                msq = prm[:, 1:2]
                var = prm[:, 2:3]
                rsd = prm[:, 3:4]
                a_ = prm[:, 4:5]
                b_ = prm[:, 5:6]
                nc.vector.tensor_scalar(
                    out=prm[:, 0:2], in0=statr[:], scalar1=inv_n,
                    scalar2=None, op0=OP.mult)
                nc.vector.tensor_tensor(var, mu, mu, OP.mult)
                nc.vector.tensor_tensor(var, msq, var, OP.subtract)
                nc.vector.tensor_scalar(
                    out=var, in0=var, scalar1=float(cfg.eps), scalar2=None,
                    op0=OP.add)
                nc.vector.reciprocal(rsd, var)
                nc.scalar.sqrt(rsd, rsd)
                nc.vector.tensor_tensor(a_, rsd, gammaT[:, l : l + 1],
                                        OP.mult)
                nc.vector.tensor_tensor(b_, mu, a_, OP.mult)
                nc.vector.tensor_tensor(b_, betaT[:, l : l + 1], b_,
                                        OP.subtract)

                # o = relu(a*agg + b), in place (halves for overlap)
                for (c0, c1) in ((0, (NT // 2) * 128),
                                 ((NT // 2) * 128, NTP)):
                    nc.scalar.activation(
                        o_fm[:, c0:c1], o_fm[:, c0:c1], ACTF.Relu,
                        bias=b_, scale=a_)
                    nc.sync.dma_start(o_out_t[l][:, c0:c1], o_fm[:, c0:c1])

    nc.compile()
    return nc


# ----------------------------------------------------------------------------
# Entry point
# ----------------------------------------------------------------------------

_CACHE = {}


def kernel(x, edge_index, Ws, bs, gammas, betas):
    import numpy as _np
    from concourse.bass_utils import run_bass_kernel_spmd

    x = _np.asarray(x, dtype=_np.float32)
    edge_index = _np.asarray(edge_index)
    Ws = _np.asarray(Ws, dtype=_np.float32)
    gammas = _np.asarray(gammas, dtype=_np.float32)
    betas = _np.asarray(betas, dtype=_np.float32)

    cfg, per_core = preprocess(x, edge_index, Ws, gammas, betas)
    key = (cfg.NT, cfg.TOTCH, tuple(int(v) for v in cfg.CH_t))
    if key not in _CACHE:
        _CACHE[key] = build_nc(cfg)
    nc = _CACHE[key]
    in_maps = [{k: _np.ascontiguousarray(v) for k, v in d.items()}
               for d in per_core]
    res = run_bass_kernel_spmd(nc, in_maps, core_ids=list(range(C)))
    core_outs = [res.results[c]["o_out"].reshape(cfg.L, 128, cfg.NTP)
                 for c in range(C)]
    return assemble_output(cfg, x, core_outs)
